# revision 1
# baseline (speedup 1.0000x reference)
"""Trainium2 Bass kernel for nn_CubicModelLarge (3-layer cubic-feature MLP).

Strategy: tensor-parallel over the cubic multiplier index i (64 values, 8 per
core).  The cubic expansion is never materialized.  Per layer:

  y[b,o] = W_lin@x + b + sum_t W_sq[o,t] xsq[b,t] + sum_i x[b,i] sum_t W_cu[o,i,t] xsq[b,t]

Rewritten per core c (i in I_c = [8c, 8c+8)):

  H[b,(il,o)] = sum_J F[J,b] * Wcub[J,(il,o)]     (one f32r GEMM, J = 2176 rows)
  y_c[b,o]    = lin[b,o] + sum_il xmac[b,il] * H[b,(il,o)]
  y = AllReduce_c(y_c)

F rows: 2048 rotation products x_a*x_{(a+d)%64} (d=0..31), 64 x rows (carries
the symmetrized W_sq fold, sharded over i via the x_i scaling), 64 gap-32
products (halved).  Rotated copies of xT are built with PE selection matmuls;
products on DVE; the i-contraction is fused scalar_tensor_tensor MACs with
per-partition scalars.  Final layer partials are summed on the host.
"""

import numpy as np

D = 64
B = 1024
NCORES = 8
I_PER = D // NCORES          # 8
OUTS = (64, 64, 10)
NKCHUNK = 16                 # rotation chunks (d pairs)
NB = B // 128                # 8 batch chunks

_CACHE = {}


# ---------------------------------------------------------------- host prep --

def _maps():
    iu, ju = np.triu_indices(D)
    tmap = np.zeros((D, D), np.int64)
    tmap[iu, ju] = np.arange(len(iu))
    tmap[ju, iu] = tmap[iu, ju]
    p = np.arange(128)
    rows_t = np.zeros((NKCHUNK, 128), np.int64)
    for k in range(NKCHUNK):
        d = 2 * k + p // 64
        a = p % 64
        rows_t[k] = tmap[a, (a + d) % D]
    d32_t = tmap[np.arange(D), (np.arange(D) + 32) % D]
    return tmap, rows_t, d32_t


def _prep_layer(W, b, out):
    """-> (wcub [NCORES](2176, I_PER*out), wlin [NCORES](65, out))"""
    _, rows_t, d32_t = _maps()
    W_lin = W[:, :D]
    W_sq = W[:, D:D + 2080]
    W_cu = W[:, D + 2080:].reshape(out, D, 2080)

    iu, ju = np.triu_indices(D)
    w2 = np.zeros((out, D, D), np.float32)
    half = np.where(iu == ju, 1.0, 0.5).astype(np.float32)
    w2[:, iu, ju] = W_sq * half
    w2[:, ju, iu] = W_sq * half

    rt = rows_t.reshape(-1)
    wcubs, wlins = [], []
    for core in range(NCORES):
        I = np.arange(core * I_PER, (core + 1) * I_PER)
        M = I_PER * out
        wcub = np.zeros((17 * 128, M), np.float32)
        blk = W_cu[:, I, :][:, :, rt]                       # (out, I_PER, 2048)
        wcub[:2048] = blk.transpose(2, 1, 0).reshape(2048, M)
        w2blk = w2[:, I, :]                                 # (out, I_PER, 64)
        wcub[2048:2048 + D] = w2blk.transpose(2, 1, 0).reshape(D, M)
        d32blk = W_cu[:, I, :][:, :, d32_t] / 2
        wcub[2048 + D:] = d32blk.transpose(2, 1, 0).reshape(D, M)
        wcubs.append(np.ascontiguousarray(wcub))

        wl = np.zeros((65, out), np.float32)
        if core == 0:
            wl[:D] = W_lin.T
            wl[D] = b
        wlins.append(wl)
    return wcubs, wlins


def _sel_consts():
    """Selection matrices, concatenated (64, (NKCHUNK+2)*128).

    slot k in 0..15: [rot_{2k}; rot_{2k+1}]   sel[c, k*128 + h*64 + a] = (c == (a + 2k + h) % 64)
    slot 16: [rot0; rot0]  (builds xT2)
    slot 17: [rot32; rot32] (first 64 cols used, builds xd32)
    """
    sel = np.zeros((D, (NKCHUNK + 2) * 128), np.float32)
    for k in range(NKCHUNK):
        for p in range(128):
            d = 2 * k + p // 64
            a = p % 64
            sel[(a + d) % D, k * 128 + p] = 1.0
    for p in range(128):
        sel[p % 64, NKCHUNK * 128 + p] = 1.0
        sel[(p % 64 + 32) % D, (NKCHUNK + 1) * 128 + p] = 1.0
    return sel


# ------------------------------------------------------------------ builder --

def _build_module():
    import concourse.bacc as bacc
    import concourse.mybir as mybir
    import concourse.tile as tile

    F32 = mybir.dt.float32
    F32R = mybir.dt.float32r
    MULT = mybir.AluOpType.mult
    ADD = mybir.AluOpType.add

    nc = bacc.Bacc("TRN2", target_bir_lowering=False, num_devices=NCORES, debug=False)

    x_in = nc.dram_tensor("x", [B, D], F32, kind="ExternalInput")
    wcub_in = [
        nc.dram_tensor(f"wcub{li}", [17 * 128, I_PER * OUTS[li]], F32, kind="ExternalInput")
        for li in range(3)
    ]
    wlin_in = [
        nc.dram_tensor(f"wlin{li}", [65, OUTS[li]], F32, kind="ExternalInput")
        for li in range(3)
    ]
    colsel_in = nc.dram_tensor("colsel", [D, I_PER], F32, kind="ExternalInput")
    out_ext = nc.dram_tensor("out", [B, OUTS[2]], F32, kind="ExternalOutput")

    sel_c = nc.inline_tensor(_sel_consts(), name="selc")
    ident_c = nc.inline_tensor(np.eye(128, dtype=np.float32), name="identc")

    with tile.TileContext(nc) as tc:
        with (
            tc.tile_pool(name="wpool", bufs=2) as wpool,
            tc.tile_pool(name="spool", bufs=1) as spool,
            tc.tile_pool(name="xpool", bufs=2) as xpool,
            tc.tile_pool(name="qpool", bufs=1) as qpool,
            tc.tile_pool(name="ypool", bufs=2) as ypool,
            tc.tile_pool(name="ps_rep", bufs=2, space="PSUM") as ps_rep,
            tc.tile_pool(name="ps_h", bufs=3, space="PSUM") as ps_h,
            tc.tile_pool(name="ps_small", bufs=3, space="PSUM") as ps_small,
            tc.tile_pool(name="dpool", bufs=2, space="DRAM") as dpool,
        ):
            sel_sb = spool.tile([D, (NKCHUNK + 2) * 128], F32R, tag="sel")
            nc.sync.dma_start(sel_sb[:], sel_c.ap().bitcast(F32R))
            ident_sb = spool.tile([128, 128], F32, tag="ident")
            nc.sync.dma_start(ident_sb[:], ident_c.ap())
            colsel_sb = spool.tile([D, I_PER], F32R, tag="colsel")
            nc.sync.dma_start(colsel_sb[:], colsel_in.ap().bitcast(F32R))

            HB = 512            # half-batch
            NBH = HB // 128     # 4 chunks per half

            # per-layer weight tiles (DMA'd up front; wpool bufs=2 double-buffers)
            weights = []
            for li in range(3):
                M = I_PER * OUTS[li]
                wcub_sb = wpool.tile([128, NKCHUNK, M], F32R, tag="wcub")
                nc.sync.dma_start(
                    wcub_sb[:],
                    wcub_in[li].ap().bitcast(F32R)[: 16 * 128, :]
                    .rearrange("(k p) m -> p k m", p=128),
                )
                wx_sb = wpool.tile([D, M], F32R, tag="wx")
                nc.sync.dma_start(wx_sb[:], wcub_in[li].ap().bitcast(F32R)[2048:2048 + D, :])
                wd32_sb = wpool.tile([D, M], F32R, tag="wd32")
                nc.sync.dma_start(wd32_sb[:], wcub_in[li].ap().bitcast(F32R)[2048 + D:, :])
                wlin_sb = wpool.tile([65, OUTS[li]], F32R, tag="wlin")
                nc.sync.dma_start(wlin_sb[:], wlin_in[li].ap().bitcast(F32R))
                weights.append((wcub_sb, wx_sb, wd32_sb, wlin_sb))

            # x tiles for layer 0, both halves, straight from the input
            x_half = []
            for h in range(2):
                xs = xpool.tile([128, NBH, D], F32, tag=f"x{h}")
                nc.sync.dma_start(
                    xs[:],
                    x_in.ap()[h * HB:(h + 1) * HB, :]
                    .rearrange("(bc p) f -> p bc f", p=128),
                )
                x_half.append(xs)

            for li in range(3):
                out_l = OUTS[li]
                M = I_PER * out_l
                last = li == 2
                wcub_sb, wx_sb, wd32_sb, wlin_sb = weights[li]
                next_x = [None, None]

                for h in range(2):
                    x_sb = x_half[h]

                    # -- phase A
                    xT_sb = xpool.tile([65, HB], F32R, tag=f"xT{h}")
                    for bc in range(NBH):
                        xTp = ps_small.tile([D, 128], F32, tag="small")
                        nc.tensor.transpose(xTp[:], x_sb[:, bc, :], ident_sb[:])
                        nc.scalar.copy(xT_sb[0:D, bc * 128:(bc + 1) * 128], xTp[:])
                    nc.vector.memset(xT_sb[D:65, :].bitcast(F32), 1.0)

                    xT2_sb = xpool.tile([128, HB], F32, tag=f"xT2{h}")
                    rep00 = ps_rep.tile([128, HB], F32, tag="rep")
                    nc.tensor.matmul(
                        rep00[:], sel_sb[:, NKCHUNK * 128:(NKCHUNK + 1) * 128],
                        xT_sb[0:D, :], start=True, stop=True,
                    )
                    nc.scalar.copy(xT2_sb[:], rep00[:])

                    xd32_sb = xpool.tile([D, HB], F32R, tag=f"xd32{h}")
                    rep32 = ps_rep.tile([128, HB], F32, tag="rep")
                    nc.tensor.matmul(
                        rep32[:], sel_sb[:, (NKCHUNK + 1) * 128:(NKCHUNK + 2) * 128],
                        xT_sb[0:D, :], start=True, stop=True,
                    )
                    nc.vector.tensor_mul(xd32_sb[:], xT2_sb[0:D, :], rep32[0:D, :])

                    # -- phase B
                    xsq = []
                    for k in range(NKCHUNK):
                        rep = ps_rep.tile([128, HB], F32, tag="rep")
                        nc.tensor.matmul(
                            rep[:], sel_sb[:, k * 128:(k + 1) * 128],
                            xT_sb[0:D, :], start=True, stop=True,
                        )
                        xq = qpool.tile([128, HB], F32R, tag=f"xsq{k}h{h}")
                        nc.vector.tensor_mul(xq[:], xT2_sb[:], rep[:])
                        xsq.append(xq)

                    # -- phase C
                    y_sb = ypool.tile([128, NBH, out_l], F32, tag=f"y{h}")
                    if not last:
                        for bc in range(NBH):
                            bs = slice(bc * 128, (bc + 1) * 128)
                            h_ps = ps_h.tile([128, M], F32, tag="h")
                            for k in range(NKCHUNK):
                                nc.tensor.matmul(
                                    h_ps[:], xsq[k][:, bs], wcub_sb[:, k, :],
                                    start=(k == 0), stop=False,
                                )
                            nc.tensor.matmul(h_ps[:], xT_sb[0:D, bs], wx_sb[:], start=False, stop=False)
                            nc.tensor.matmul(h_ps[:], xd32_sb[:, bs], wd32_sb[:], start=False, stop=True)

                            lin_ps = ps_small.tile([128, out_l], F32, tag="small")
                            nc.tensor.matmul(lin_ps[:], xT_sb[0:65, bs], wlin_sb[:], start=True, stop=True)
                            xmac_ps = ps_small.tile([128, I_PER], F32, tag="small")
                            nc.tensor.matmul(xmac_ps[:], xT_sb[0:D, bs], colsel_sb[:], start=True, stop=True)
                            xmac_sb = ypool.tile([128, I_PER], F32, tag="xmac")
                            nc.scalar.copy(xmac_sb[:], xmac_ps[:])

                            nc.scalar.copy(y_sb[:, bc, :], lin_ps[:])
                            for il in range(I_PER):
                                nc.vector.scalar_tensor_tensor(
                                    y_sb[:, bc, :],
                                    h_ps[:, il * out_l:(il + 1) * out_l],
                                    xmac_sb[:, il:il + 1],
                                    y_sb[:, bc, :],
                                    op0=MULT, op1=ADD,
                                )

                        # -- phase D: AllReduce this half
                        y_bounce = dpool.tile([HB, out_l], F32, tag=f"ybounce{h}")
                        y_red = dpool.tile([HB, out_l], F32, tag=f"yred{h}")
                        nc.sync.dma_start(
                            y_bounce[:].rearrange("(bc p) o -> p bc o", p=128), y_sb[:]
                        )
                        nc.gpsimd.collective_compute(
                            "AllReduce",
                            ADD,
                            replica_groups=[list(range(NCORES))],
                            ins=[y_bounce.opt()],
                            outs=[y_red.opt()],
                        )
                        xs = xpool.tile([128, NBH, D], F32, tag=f"x{h}")
                        nc.sync.dma_start(
                            xs[:], y_red[:].rearrange("(bc p) f -> p bc f", p=128)
                        )
                        next_x[h] = xs
                    else:
                        # layer 2: stationary-W GEMM, transpose, MAC
                        h_ps = ps_h.tile([M, HB], F32, tag="h")
                        for k in range(NKCHUNK):
                            nc.tensor.matmul(
                                h_ps[:], wcub_sb[:, k, :], xsq[k][:],
                                start=(k == 0), stop=False,
                            )
                        nc.tensor.matmul(h_ps[:], wx_sb[:], xT_sb[0:D, :], start=False, stop=False)
                        nc.tensor.matmul(h_ps[:], wd32_sb[:], xd32_sb[:], start=False, stop=True)
                        h2_sb = ypool.tile([M, HB], F32, tag=f"h2{h}")
                        nc.scalar.copy(h2_sb[:], h_ps[:])

                        for bc in range(NBH):
                            bs = slice(bc * 128, (bc + 1) * 128)
                            h2t_ps = ps_small.tile([128, M], F32, tag="small")
                            nc.tensor.transpose(h2t_ps[:], h2_sb[:, bs], ident_sb[0:M, 0:M])

                            lin_ps = ps_small.tile([128, out_l], F32, tag="small")
                            nc.tensor.matmul(lin_ps[:], xT_sb[0:65, bs], wlin_sb[:], start=True, stop=True)
                            xmac_ps = ps_small.tile([128, I_PER], F32, tag="small")
                            nc.tensor.matmul(xmac_ps[:], xT_sb[0:D, bs], colsel_sb[:], start=True, stop=True)
                            xmac_sb = ypool.tile([128, I_PER], F32, tag="xmac")
                            nc.scalar.copy(xmac_sb[:], xmac_ps[:])

                            nc.scalar.copy(y_sb[:, bc, :], lin_ps[:])
                            for il in range(I_PER):
                                nc.vector.scalar_tensor_tensor(
                                    y_sb[:, bc, :],
                                    h2t_ps[:, il * out_l:(il + 1) * out_l],
                                    xmac_sb[:, il:il + 1],
                                    y_sb[:, bc, :],
                                    op0=MULT, op1=ADD,
                                )

                        nc.sync.dma_start(
                            out_ext.ap()[h * HB:(h + 1) * HB, :]
                            .rearrange("(bc p) o -> p bc o", p=128),
                            y_sb[:],
                        )

                if not last:
                    x_half = next_x

    nc.compile()
    return nc


# ------------------------------------------------------------------- runner --

def kernel(x, W0, b0, W1, b1, W2, b2):
    from concourse.bass_utils import run_bass_kernel_spmd

    if "nc" not in _CACHE:
        _CACHE["nc"] = _build_module()
    nc = _CACHE["nc"]

    x = np.ascontiguousarray(np.asarray(x, np.float32))
    Ws = [np.asarray(W, np.float32) for W in (W0, W1, W2)]
    bs = [np.asarray(b_, np.float32) for b_ in (b0, b1, b2)]

    wcubs, wlins = {}, {}
    for li in range(3):
        wcubs[li], wlins[li] = _prep_layer(Ws[li], bs[li], OUTS[li])

    in_maps = []
    for core in range(NCORES):
        I = np.arange(core * I_PER, (core + 1) * I_PER)
        colsel = np.zeros((D, I_PER), np.float32)
        colsel[I, np.arange(I_PER)] = 1.0
        m = {"x": x, "colsel": colsel}
        for li in range(3):
            m[f"wcub{li}"] = wcubs[li][core]
            m[f"wlin{li}"] = wlins[li][core]
        in_maps.append(m)

    res = run_bass_kernel_spmd(nc, in_maps, core_ids=list(range(NCORES)))
    out = np.zeros((B, OUTS[2]), np.float32)
    for core in range(NCORES):
        out += res.results[core]["out"]
    return out



# revision 2
# speedup vs baseline: 1.4522x; 1.4522x over previous
"""Trainium2 Bass kernel for nn_CubicModelLarge (3-layer cubic-feature MLP).

Strategy: tensor-parallel over the cubic multiplier index i (64 values, 8 per
core).  The cubic expansion is never materialized.  Per layer:

  y[b,o] = W_lin@x + b + sum_t W_sq[o,t] xsq[b,t] + sum_i x[b,i] sum_t W_cu[o,i,t] xsq[b,t]

Rewritten per core c (i in I_c = [8c, 8c+8)):

  H[b,(il,o)] = sum_J F[J,b] * Wcub[J,(il,o)]     (one bf16 GEMM, J = 2176 rows)
  y_c[b,o]    = lin[b,o] + sum_il xmac[b,il] * H[b,(il,o)]
  y = AllReduce_c(y_c)

F rows: 2048 rotation products x_a*x_{(a+d)%64} (d=0..31), 64 x rows (carries
the symmetrized W_sq fold, sharded over i via the x_i scaling), 64 gap-32
products (halved).  Rotated copies of xT are built with PE selection matmuls;
products on DVE; the i-contraction is fused scalar_tensor_tensor MACs with
per-partition scalars.  Final layer partials are summed on the host.

v1 perf changes vs baseline:
  - all heavy GEMM operands in bf16 (weights cast on host; xsq products
    rounded once on DVE output) -> 1 cyc/row matmuls, half the weight DMA
  - x-rows and d32-rows merged into wcub chunk 16 -> 17 matmuls per bc
  - per-layer weight tiles all resident (bf16 fits) -> no pool-reuse stalls
  - weight DMA on the ACT HWDGE ring, latency-critical DMA on the SP ring
    -> the AllReduce bounce is no longer head-of-line blocked
"""

import numpy as np

D = 64
B = 1024
NCORES = 8
I_PER = D // NCORES          # 8
OUTS = (64, 64, 10)
NKCHUNK = 16                 # rotation chunks (d pairs)
NCHUNK = 17                  # + the [x; d32] chunk
NB = B // 128                # 8 batch chunks

_CACHE = {}


def _bf16():
    import concourse.mybir as mybir
    return mybir.dt.np(mybir.dt.bfloat16)


# ---------------------------------------------------------------- host prep --

def _maps():
    iu, ju = np.triu_indices(D)
    tmap = np.zeros((D, D), np.int64)
    tmap[iu, ju] = np.arange(len(iu))
    tmap[ju, iu] = tmap[iu, ju]
    p = np.arange(128)
    rows_t = np.zeros((NKCHUNK, 128), np.int64)
    for k in range(NKCHUNK):
        d = 2 * k + p // 64
        a = p % 64
        rows_t[k] = tmap[a, (a + d) % D]
    d32_t = tmap[np.arange(D), (np.arange(D) + 32) % D]
    return tmap, rows_t, d32_t


def _prep_layer(W, b, out):
    """-> (wcub [NCORES](2176, I_PER*out) bf16, wlin [NCORES](65, out) bf16)"""
    _, rows_t, d32_t = _maps()
    bf16 = _bf16()
    W_lin = W[:, :D]
    W_sq = W[:, D:D + 2080]
    W_cu = W[:, D + 2080:].reshape(out, D, 2080)

    iu, ju = np.triu_indices(D)
    w2 = np.zeros((out, D, D), np.float32)
    half = np.where(iu == ju, 1.0, 0.5).astype(np.float32)
    w2[:, iu, ju] = W_sq * half
    w2[:, ju, iu] = W_sq * half

    rt = rows_t.reshape(-1)
    wcubs, wlins = [], []
    for core in range(NCORES):
        I = np.arange(core * I_PER, (core + 1) * I_PER)
        M = I_PER * out
        wcub = np.zeros((NCHUNK * 128, M), np.float32)
        blk = W_cu[:, I, :][:, :, rt]                       # (out, I_PER, 2048)
        wcub[:2048] = blk.transpose(2, 1, 0).reshape(2048, M)
        w2blk = w2[:, I, :]                                 # (out, I_PER, 64)
        wcub[2048:2048 + D] = w2blk.transpose(2, 1, 0).reshape(D, M)
        d32blk = W_cu[:, I, :][:, :, d32_t] / 2
        wcub[2048 + D:] = d32blk.transpose(2, 1, 0).reshape(D, M)
        wcubs.append(np.ascontiguousarray(wcub.astype(bf16)))

        wl = np.zeros((65, out), np.float32)
        if core == 0:
            wl[:D] = W_lin.T
            wl[D] = b
        wlins.append(wl.astype(bf16))
    return wcubs, wlins


def _sel_consts():
    """Selection matrices, concatenated (64, (NKCHUNK+2)*128), bf16.

    slot k in 0..15: [rot_{2k}; rot_{2k+1}]   sel[c, k*128 + h*64 + a] = (c == (a + 2k + h) % 64)
    slot 16: [rot0; rot0]  (builds xT2)
    slot 17: [rot32; rot32] (first 64 cols used, builds xd32)
    """
    sel = np.zeros((D, (NKCHUNK + 2) * 128), np.float32)
    for k in range(NKCHUNK):
        for p in range(128):
            d = 2 * k + p // 64
            a = p % 64
            sel[(a + d) % D, k * 128 + p] = 1.0
    for p in range(128):
        sel[p % 64, NKCHUNK * 128 + p] = 1.0
        sel[(p % 64 + 32) % D, (NKCHUNK + 1) * 128 + p] = 1.0
    return sel.astype(_bf16())


def _in_maps(x, Ws, bs):
    """Build the per-core input maps (shared by kernel() and test.py)."""
    bf16 = _bf16()
    wcubs, wlins = {}, {}
    for li in range(3):
        wcubs[li], wlins[li] = _prep_layer(Ws[li], bs[li], OUTS[li])

    in_maps = []
    for core in range(NCORES):
        I = np.arange(core * I_PER, (core + 1) * I_PER)
        colsel = np.zeros((D, I_PER), np.float32)
        colsel[I, np.arange(I_PER)] = 1.0
        m = {"x": x, "colsel": colsel.astype(bf16)}
        for li in range(3):
            m[f"wcub{li}"] = wcubs[li][core]
            m[f"wlin{li}"] = wlins[li][core]
        in_maps.append(m)
    return in_maps


# ------------------------------------------------------------------ builder --

def _build_module():
    import concourse.bacc as bacc
    import concourse.mybir as mybir
    import concourse.tile as tile

    F32 = mybir.dt.float32
    BF16 = mybir.dt.bfloat16
    MULT = mybir.AluOpType.mult
    ADD = mybir.AluOpType.add

    nc = bacc.Bacc("TRN2", target_bir_lowering=False, num_devices=NCORES, debug=False)

    x_in = nc.dram_tensor("x", [B, D], F32, kind="ExternalInput")
    wcub_in = [
        nc.dram_tensor(f"wcub{li}", [NCHUNK * 128, I_PER * OUTS[li]], BF16, kind="ExternalInput")
        for li in range(3)
    ]
    wlin_in = [
        nc.dram_tensor(f"wlin{li}", [65, OUTS[li]], BF16, kind="ExternalInput")
        for li in range(3)
    ]
    colsel_in = nc.dram_tensor("colsel", [D, I_PER], BF16, kind="ExternalInput")
    out_ext = nc.dram_tensor("out", [B, OUTS[2]], F32, kind="ExternalOutput")

    sel_c = nc.inline_tensor(_sel_consts(), name="selc")
    ident_c = nc.inline_tensor(np.eye(128, dtype=np.float32), name="identc")

    with tile.TileContext(nc) as tc:
        with (
            tc.tile_pool(name="wpool", bufs=1) as wpool,
            tc.tile_pool(name="spool", bufs=1) as spool,
            tc.tile_pool(name="xpool", bufs=2) as xpool,
            tc.tile_pool(name="qpool", bufs=1) as qpool,
            tc.tile_pool(name="ypool", bufs=2) as ypool,
            tc.tile_pool(name="ps_rep", bufs=2, space="PSUM") as ps_rep,
            tc.tile_pool(name="ps_h", bufs=3, space="PSUM") as ps_h,
            tc.tile_pool(name="ps_small", bufs=3, space="PSUM") as ps_small,
            tc.tile_pool(name="dpool", bufs=2, space="DRAM") as dpool,
        ):
            sel_sb = spool.tile([D, (NKCHUNK + 2) * 128], BF16, tag="sel")
            nc.scalar.dma_start(sel_sb[:], sel_c.ap())
            ident_sb = spool.tile([128, 128], F32, tag="ident")
            nc.scalar.dma_start(ident_sb[:], ident_c.ap())
            colsel_sb = spool.tile([D, I_PER], BF16, tag="colsel")
            nc.scalar.dma_start(colsel_sb[:], colsel_in.ap())

            HB = 512            # half-batch
            NBH = HB // 128     # 4 chunks per half

            # per-layer weight tiles, all resident (bf16), on the ACT DMA ring
            weights = []
            for li in range(3):
                M = I_PER * OUTS[li]
                wcub_sb = wpool.tile([128, NCHUNK, M], BF16, tag=f"wcub{li}")
                nc.scalar.dma_start(
                    wcub_sb[:],
                    wcub_in[li].ap().rearrange("(k p) m -> p k m", p=128),
                )
                wlin_sb = wpool.tile([65, OUTS[li]], BF16, tag=f"wlin{li}")
                nc.scalar.dma_start(wlin_sb[:], wlin_in[li].ap())
                weights.append((wcub_sb, wlin_sb))

            # x tiles for layer 0, both halves, straight from the input
            x_half = []
            for h in range(2):
                xs = xpool.tile([128, NBH, D], F32, tag=f"x{h}")
                nc.sync.dma_start(
                    xs[:],
                    x_in.ap()[h * HB:(h + 1) * HB, :]
                    .rearrange("(bc p) f -> p bc f", p=128),
                )
                x_half.append(xs)

            for li in range(3):
                out_l = OUTS[li]
                M = I_PER * out_l
                last = li == 2
                wcub_sb, wlin_sb = weights[li]
                next_x = [None, None]

                for h in range(2):
                    x_sb = x_half[h]

                    # -- phase A: transpose x -> xTb (bf16) and xstack rows 0:64
                    xTb = xpool.tile([65, HB], BF16, tag=f"xT{h}")
                    xstack = xpool.tile([128, HB], BF16, tag=f"xstack{h}")
                    for bc in range(NBH):
                        xTp = ps_small.tile([D, 128], F32, tag="small")
                        nc.tensor.transpose(xTp[:], x_sb[:, bc, :], ident_sb[:])
                        nc.scalar.copy(xTb[0:D, bc * 128:(bc + 1) * 128], xTp[:])
                        nc.scalar.copy(xstack[0:D, bc * 128:(bc + 1) * 128], xTp[:])
                    nc.vector.memset(xTb[D:65, :], 1.0)

                    xT2_sb = xpool.tile([128, HB], BF16, tag=f"xT2{h}")
                    rep00 = ps_rep.tile([128, HB], F32, tag="rep")
                    nc.tensor.matmul(
                        rep00[:], sel_sb[:, NKCHUNK * 128:(NKCHUNK + 1) * 128],
                        xTb[0:D, :], start=True, stop=True,
                    )
                    nc.scalar.copy(xT2_sb[:], rep00[:])

                    # xstack rows 64:128 = xd32 = x * rot32(x), halved in W
                    rep32 = ps_rep.tile([128, HB], F32, tag="rep")
                    nc.tensor.matmul(
                        rep32[:], sel_sb[:, (NKCHUNK + 1) * 128:(NKCHUNK + 2) * 128],
                        xTb[0:D, :], start=True, stop=True,
                    )
                    nc.vector.tensor_mul(xstack[D:128, :], xT2_sb[0:D, :], rep32[0:D, :])

                    # -- phase B: rotation products -> xsq chunks (bf16)
                    xsq = []
                    for k in range(NKCHUNK):
                        rep = ps_rep.tile([128, HB], F32, tag="rep")
                        nc.tensor.matmul(
                            rep[:], sel_sb[:, k * 128:(k + 1) * 128],
                            xTb[0:D, :], start=True, stop=True,
                        )
                        xq = qpool.tile([128, HB], BF16, tag=f"xsq{k}h{h}")
                        nc.vector.tensor_mul(xq[:], xT2_sb[:], rep[:])
                        xsq.append(xq)
                    xsq.append(xstack)

                    # -- phase C
                    y_sb = ypool.tile([128, NBH, out_l], F32, tag=f"y{h}")
                    if not last:
                        for bc in range(NBH):
                            bs = slice(bc * 128, (bc + 1) * 128)
                            h_ps = ps_h.tile([128, M], F32, tag="h")
                            for k in range(NCHUNK):
                                nc.tensor.matmul(
                                    h_ps[:], xsq[k][:, bs], wcub_sb[:, k, :],
                                    start=(k == 0), stop=(k == NCHUNK - 1),
                                )

                            lin_ps = ps_small.tile([128, out_l], F32, tag="small")
                            nc.tensor.matmul(lin_ps[:], xTb[:, bs], wlin_sb[:], start=True, stop=True)
                            xmac_ps = ps_small.tile([128, I_PER], F32, tag="small")
                            nc.tensor.matmul(xmac_ps[:], xTb[0:D, bs], colsel_sb[:], start=True, stop=True)
                            xmac_sb = ypool.tile([128, I_PER], F32, tag="xmac")
                            nc.scalar.copy(xmac_sb[:], xmac_ps[:])

                            nc.scalar.copy(y_sb[:, bc, :], lin_ps[:])
                            for il in range(I_PER):
                                nc.vector.scalar_tensor_tensor(
                                    y_sb[:, bc, :],
                                    h_ps[:, il * out_l:(il + 1) * out_l],
                                    xmac_sb[:, il:il + 1],
                                    y_sb[:, bc, :],
                                    op0=MULT, op1=ADD,
                                )

                        # -- phase D: AllReduce this half
                        y_bounce = dpool.tile([HB, out_l], F32, tag=f"ybounce{h}")
                        y_red = dpool.tile([HB, out_l], F32, tag=f"yred{h}")
                        nc.sync.dma_start(
                            y_bounce[:].rearrange("(bc p) o -> p bc o", p=128), y_sb[:]
                        )
                        nc.gpsimd.collective_compute(
                            "AllReduce",
                            ADD,
                            replica_groups=[list(range(NCORES))],
                            ins=[y_bounce.opt()],
                            outs=[y_red.opt()],
                        )
                        xs = xpool.tile([128, NBH, D], F32, tag=f"x{h}")
                        nc.sync.dma_start(
                            xs[:], y_red[:].rearrange("(bc p) f -> p bc f", p=128)
                        )
                        next_x[h] = xs
                    else:
                        # layer 2: stationary-W GEMM, transpose, MAC
                        h_ps = ps_h.tile([M, HB], F32, tag="h")
                        for k in range(NCHUNK):
                            nc.tensor.matmul(
                                h_ps[:], wcub_sb[:, k, 0:M], xsq[k][:],
                                start=(k == 0), stop=(k == NCHUNK - 1),
                            )
                        h2_sb = ypool.tile([M, HB], F32, tag=f"h2{h}")
                        nc.scalar.copy(h2_sb[:], h_ps[:])

                        for bc in range(NBH):
                            bs = slice(bc * 128, (bc + 1) * 128)
                            h2t_ps = ps_small.tile([128, M], F32, tag="small")
                            nc.tensor.transpose(h2t_ps[:], h2_sb[:, bs], ident_sb[0:M, 0:M])

                            lin_ps = ps_small.tile([128, out_l], F32, tag="small")
                            nc.tensor.matmul(lin_ps[:], xTb[:, bs], wlin_sb[:], start=True, stop=True)
                            xmac_ps = ps_small.tile([128, I_PER], F32, tag="small")
                            nc.tensor.matmul(xmac_ps[:], xTb[0:D, bs], colsel_sb[:], start=True, stop=True)
                            xmac_sb = ypool.tile([128, I_PER], F32, tag="xmac")
                            nc.scalar.copy(xmac_sb[:], xmac_ps[:])

                            nc.scalar.copy(y_sb[:, bc, :], lin_ps[:])
                            for il in range(I_PER):
                                nc.vector.scalar_tensor_tensor(
                                    y_sb[:, bc, :],
                                    h2t_ps[:, il * out_l:(il + 1) * out_l],
                                    xmac_sb[:, il:il + 1],
                                    y_sb[:, bc, :],
                                    op0=MULT, op1=ADD,
                                )

                        nc.sync.dma_start(
                            out_ext.ap()[h * HB:(h + 1) * HB, :]
                            .rearrange("(bc p) o -> p bc o", p=128),
                            y_sb[:],
                        )

                if not last:
                    x_half = next_x

    nc.compile()
    return nc


# ------------------------------------------------------------------- runner --

def kernel(x, W0, b0, W1, b1, W2, b2):
    from concourse.bass_utils import run_bass_kernel_spmd

    if "nc" not in _CACHE:
        _CACHE["nc"] = _build_module()
    nc = _CACHE["nc"]

    x = np.ascontiguousarray(np.asarray(x, np.float32))
    Ws = [np.asarray(W, np.float32) for W in (W0, W1, W2)]
    bs = [np.asarray(b_, np.float32) for b_ in (b0, b1, b2)]

    in_maps = _in_maps(x, Ws, bs)
    res = run_bass_kernel_spmd(nc, in_maps, core_ids=list(range(NCORES)))
    out = np.zeros((B, OUTS[2]), np.float32)
    for core in range(NCORES):
        out += res.results[core]["out"]
    return out


# revision 6
# speedup vs baseline: 1.7706x; 1.2193x over previous
"""Trainium2 Bass kernel for nn_CubicModelLarge (3-layer cubic-feature MLP).

v2: output-sharded tensor parallelism in fp16, AllGather boundaries.

Per layer, each core owns a contiguous block of 8 outputs (layer 2: 10
outputs padded to 16, 2 per core, host concatenates).  The cubic expansion is
never materialized; per core:

  H[(o,i), b] = sum_J Wcub[J, (o,i)] * F[J, b]      (17x 128-row GEMM chunks)
  y^T[o, b]   = lin[o, b] + sum_i x[b,i] * H[(o,i), b]

F rows (2176 = 17*128): 2048 rotation products x_a*x_{(a+d)%64} (d=0..31),
64 x rows (carry the symmetrized W_sq fold), 64 gap-32 products (halved).

Rotations exploit xT2 = [x;x]: rotation-by-d is a contiguous 64-partition
window of xT2, built with 2 small SBUF->SBUF DMAs per chunk (no PE selection
matmuls, no PSUM roundtrip).  Products run on DVE in fp16 2x mode.  The
i-contraction is an elementwise multiply by xT2 (DVE) plus 0/1 segment-sum
matmuls on PE accumulating straight into y^T PSUM along with the linear term.

Each core computes the full i-range, so result slices are complete: the layer
boundary is an AllGather of y^T (8, 512) fp16 whose output (64, 512) IS the
next layer's xT (x is fed in pre-transposed).  No AllReduce anywhere.

All matmul operands fp16 (1 cyc/row); PSUM accumulation fp32.  Weight DMA on
the ACT HWDGE ring; latency-critical DMA on the SP ring.
"""

import numpy as np

D = 64
B = 1024
NCORES = 8
OUTS = (64, 64, 10)
OPER = (8, 8, 2)             # outputs per core (layer 2 padded to 16)
NKCHUNK = 16                 # rotation chunks (d pairs)
NCHUNK = 17                  # + the [x; d32] chunk
HB = 512                     # half-batch

_CACHE = {}


# ---------------------------------------------------------------- host prep --

def _maps():
    iu, ju = np.triu_indices(D)
    tmap = np.zeros((D, D), np.int64)
    tmap[iu, ju] = np.arange(len(iu))
    tmap[ju, iu] = tmap[iu, ju]
    p = np.arange(128)
    rows_t = np.zeros((NKCHUNK, 128), np.int64)
    for k in range(NKCHUNK):
        d = 2 * k + p // 64
        a = p % 64
        rows_t[k] = tmap[a, (a + d) % D]
    d32_t = tmap[np.arange(D), (np.arange(D) + 32) % D]
    return tmap, rows_t, d32_t


def _prep_layer(W, b, oper):
    """-> (wcub [NCORES](2176, oper*64) f16, wlin [NCORES](65, oper) f16)

    m = o_loc*64 + i ordering (o-major) so the i segment-sum is a 64-run."""
    _, rows_t, d32_t = _maps()
    out = W.shape[0]
    W_lin = W[:, :D]
    W_sq = W[:, D:D + 2080]
    W_cu = W[:, D + 2080:].reshape(out, D, 2080)

    iu, ju = np.triu_indices(D)
    w2 = np.zeros((out, D, D), np.float32)
    half = np.where(iu == ju, 1.0, 0.5).astype(np.float32)
    w2[:, iu, ju] = W_sq * half
    w2[:, ju, iu] = W_sq * half

    rt = rows_t.reshape(-1)
    wcubs, wlins = [], []
    for core in range(NCORES):
        o_lo = core * oper
        o_sl = [o for o in range(o_lo, o_lo + oper) if o < out]
        n_real = len(o_sl)
        M = oper * D
        wcub = np.zeros((NCHUNK * 128, M), np.float32)
        wl = np.zeros((65, oper), np.float32)
        if n_real:
            blk = W_cu[o_sl, :, :][:, :, rt]                # (n, 64, 2048)
            wcub[:2048, :n_real * D] = blk.transpose(2, 0, 1).reshape(2048, n_real * D)
            w2blk = w2[o_sl]                                # (n, 64i, 64f)
            wcub[2048:2048 + D, :n_real * D] = w2blk.transpose(2, 0, 1).reshape(D, n_real * D)
            d32blk = W_cu[o_sl, :, :][:, :, d32_t] / 2      # (n, 64i, 64a)
            wcub[2048 + D:, :n_real * D] = d32blk.transpose(2, 0, 1).reshape(D, n_real * D)
            wl[:D, :n_real] = W_lin[o_sl].T
            wl[D, :n_real] = b[o_sl]
        wcubs.append(np.ascontiguousarray(wcub.astype(np.float16)))
        wlins.append(wl.astype(np.float16))
    return wcubs, wlins


def _seg_consts():
    """(128, 32) fp16: S[p, 10*s + p//64] = 1 (cols [8s:8s+8] = slice s).

    lhsT for the i segment-sum: slice s maps tmp_s partitions q*64+i to y
    rows {2s, 2s+1}.  Columns 0:2 double as the layer-2 (oper=2) matrix."""
    S = np.zeros((128, 32), np.float32)
    for p in range(128):
        for s in range(4):
            S[p, 8 * s + 2 * s + p // 64] = 1.0
    return S.astype(np.float16)


def _in_maps(x, Ws, bs):
    """Build the per-core input maps (shared by kernel() and test.py)."""
    xT16 = np.ascontiguousarray(np.asarray(x, np.float32).astype(np.float16).T)
    wcubs, wlins = {}, {}
    for li in range(3):
        wcubs[li], wlins[li] = _prep_layer(Ws[li], bs[li], OPER[li])

    in_maps = []
    for core in range(NCORES):
        m = {"x": xT16}
        for li in range(3):
            m[f"wcub{li}"] = wcubs[li][core]
            m[f"wlin{li}"] = wlins[li][core]
        in_maps.append(m)
    return in_maps


# ------------------------------------------------------------------ builder --

def _build_module():
    import concourse.bacc as bacc
    import concourse.mybir as mybir
    import concourse.tile as tile

    F32 = mybir.dt.float32
    F16 = mybir.dt.float16
    BYPASS = mybir.AluOpType.bypass

    nc = bacc.Bacc("TRN2", target_bir_lowering=False, num_devices=NCORES, debug=False)

    x_in = nc.dram_tensor("x", [D, B], F16, kind="ExternalInput")
    wcub_in = [
        nc.dram_tensor(f"wcub{li}", [NCHUNK * 128, OPER[li] * D], F16, kind="ExternalInput")
        for li in range(3)
    ]
    wlin_in = [
        nc.dram_tensor(f"wlin{li}", [65, OPER[li]], F16, kind="ExternalInput")
        for li in range(3)
    ]
    out_ext = nc.dram_tensor("out", [OPER[2], B], F32, kind="ExternalOutput")

    seg_c = nc.inline_tensor(_seg_consts(), name="segc")

    with tile.TileContext(nc) as tc:
        with (
            tc.tile_pool(name="wpool", bufs=1) as wpool,
            tc.tile_pool(name="spool", bufs=1) as spool,
            tc.tile_pool(name="xpool", bufs=2) as xpool,
            tc.tile_pool(name="qpool", bufs=1) as qpool,
            tc.tile_pool(name="tpool", bufs=3) as tpool,
            tc.tile_pool(name="ypool", bufs=2) as ypool,
            tc.tile_pool(name="ps_h", bufs=4, space="PSUM") as ps_h,
            tc.tile_pool(name="ps_y", bufs=2, space="PSUM") as ps_y,
            tc.tile_pool(name="dpool", bufs=2, space="DRAM") as dpool,
        ):
            seg_sb = spool.tile([128, 32], F16, tag="seg")
            nc.scalar.dma_start(seg_sb[:], seg_c.ap())

            # per-layer weight tiles, all resident, on the ACT DMA ring
            weights = []
            for li in range(3):
                M = OPER[li] * D
                wcub_sb = wpool.tile([128, NCHUNK, M], F16, tag=f"wcub{li}")
                nc.scalar.dma_start(
                    wcub_sb[:],
                    wcub_in[li].ap().rearrange("(k p) m -> p k m", p=128),
                )
                wlin_sb = wpool.tile([65, OPER[li]], F16, tag=f"wlin{li}")
                nc.scalar.dma_start(wlin_sb[:], wlin_in[li].ap())
                weights.append((wcub_sb, wlin_sb))

            # x arrives pre-transposed: (64, B) in DRAM
            xsrc = [x_in.ap()[:, 0:HB], x_in.ap()[:, HB:B]]

            for li in range(3):
                oper = OPER[li]
                nsl = (oper * D) // 128          # m slices: 4, 4, 1
                last = li == 2
                wcub_sb, wlin_sb = weights[li]

                # ---- phase A+B for both halves: x views and products
                xTb = [None, None]
                xT2 = [None, None]
                xsq = [[], []]
                for h in range(2):
                    xt = xpool.tile([65, HB], F16, tag=f"xT{li&1}{h}")
                    nc.sync.dma_start(xt[0:D, :], xsrc[h])
                    nc.vector.memset(xt[D:65, :], 1.0)
                    xTb[h] = xt

                    x2 = xpool.tile([128, HB], F16, tag=f"xT2{li&1}{h}")
                    nc.sync.dma_start(x2[0:D, :], xsrc[h])
                    nc.sync.dma_start(x2[D:128, :], xsrc[h])
                    xT2[h] = x2

                    xstack = xpool.tile([128, HB], F16, tag=f"xstack{li&1}{h}")
                    nc.sync.dma_start(xstack[0:D, :], xsrc[h])
                    # gap-32 products (halved in the weights); engine operands
                    # cannot straddle 64-partition blocks, so bounce via DMA
                    rot32 = xpool.tile([D, HB], F16, tag=f"rot32{li&1}{h}")
                    nc.sync.dma_start(rot32[:], x2[32:96, :])
                    nc.vector.tensor_mul(xstack[D:128, :], x2[0:D, :], rot32[:])

                    for k in range(NKCHUNK):
                        rot = qpool.tile([128, HB], F16, tag=f"rot{k}h{h}")
                        nc.sync.dma_start(rot[0:D, :], x2[2 * k:2 * k + D, :])
                        nc.sync.dma_start(rot[D:128, :], x2[2 * k + 1:2 * k + 1 + D, :])
                        xq = qpool.tile([128, HB], F16, tag=f"xsq{k}h{h}")
                        nc.vector.tensor_mul(xq[:], x2[:], rot[:])
                        xsq[h].append(xq)
                    xsq[h].append(xstack)

                # ---- phase C + segment-sum + boundary, per half
                nxt = [None, None]
                for h in range(2):
                    y_ps = ps_y.tile([oper, HB], F32, tag="y")
                    nc.tensor.matmul(
                        y_ps[:], wlin_sb[:], xTb[h][:],
                        start=True, stop=False, skip_group_check=True,
                    )
                    for s in range(nsl):
                        h_ps = ps_h.tile([128, HB], F32, tag="h")
                        for k in range(NCHUNK):
                            nc.tensor.matmul(
                                h_ps[:], wcub_sb[:, k, 128 * s:128 * (s + 1)],
                                xsq[h][k][:], start=(k == 0), stop=(k == NCHUNK - 1),
                            )
                        tmp = tpool.tile([128, HB], F16, tag="tmp")
                        nc.vector.tensor_mul(tmp[:], h_ps[:], xT2[h][:])
                        nc.tensor.matmul(
                            y_ps[:], seg_sb[:, 8 * s:8 * s + oper],
                            tmp[:], start=False, stop=(s == nsl - 1),
                            skip_group_check=True,
                        )

                    if not last:
                        y_sb = ypool.tile([oper, HB], F16, tag=f"y{h}")
                        nc.scalar.copy(y_sb[:], y_ps[:])
                        y_bounce = dpool.tile([oper, HB], F16, tag=f"yb{li}{h}")
                        yg = dpool.tile([D, HB], F16, tag=f"yg{li}{h}")
                        nc.sync.dma_start(y_bounce[:], y_sb[:])
                        nc.gpsimd.collective_compute(
                            "AllGather",
                            BYPASS,
                            replica_groups=[list(range(NCORES))],
                            ins=[y_bounce.opt()],
                            outs=[yg.opt()],
                        )
                        nxt[h] = yg
                    else:
                        y_sb = ypool.tile([oper, HB], F32, tag=f"yf{h}")
                        nc.scalar.copy(y_sb[:], y_ps[:])
                        nc.sync.dma_start(
                            out_ext.ap()[:, h * HB:(h + 1) * HB], y_sb[:]
                        )

                if not last:
                    xsrc = [nxt[0][:], nxt[1][:]]

    nc.compile()
    return nc


# ------------------------------------------------------------------- runner --

def kernel(x, W0, b0, W1, b1, W2, b2):
    from concourse.bass_utils import run_bass_kernel_spmd

    if "nc" not in _CACHE:
        _CACHE["nc"] = _build_module()
    nc = _CACHE["nc"]

    Ws = [np.asarray(W, np.float32) for W in (W0, W1, W2)]
    bs = [np.asarray(b_, np.float32) for b_ in (b0, b1, b2)]

    in_maps = _in_maps(x, Ws, bs)
    res = run_bass_kernel_spmd(nc, in_maps, core_ids=list(range(NCORES)))
    out = np.zeros((B, OUTS[2]), np.float32)
    for core in range(5):
        o_lo = core * OPER[2]
        n = min(OPER[2], OUTS[2] - o_lo)
        out[:, o_lo:o_lo + n] = res.results[core]["out"][:n, :].T
    return out
